# revision 4
# baseline (speedup 1.0000x reference)
"""Trainium2 Bass kernel for nn_ExternalInteraction_9079560863791.

Computes, per batch row b:
    out_user[b, :]  = user_attributes[b, :]  * sum(image_attributes[b, :])
    out_image[b, :] = image_attributes[b, :] * sum(user_attributes[b, :])

Pure data parallel over the batch axis: 2048 rows split across 8 NeuronCores
(256 rows each). Per core: 2 blocks of 128 rows; each block loads a
[128, 4096] f32 tile per tensor, row-sums on the vector engine, and applies
the per-partition broadcast multiply (DVE tensor_scalar for one output, ACT
scaled-copy for the other, to spread compute across engines). Memory-bound:
~16 MiB of HBM traffic per core -> ~47 us roofline at ~358 GB/s.
"""

import sys

for _p in ("/opt/trn_rl_repo", "/opt/pypackages"):
    if _p not in sys.path:
        sys.path.append(_p)

import numpy as np

N_CORES = 8
B, D = 2048, 4096
ROWS = B // N_CORES  # 256 rows per core
P = 128  # SBUF partitions
N_BLOCKS = ROWS // P  # 2 blocks per core

_CACHE = {}


def _build():
    import concourse.tile as tile
    from concourse import bacc, mybir

    nc = bacc.Bacc(
        "TRN2",
        target_bir_lowering=False,
        debug=False,
        enable_asserts=False,
        num_devices=N_CORES,
    )
    f32 = mybir.dt.float32

    u = nc.dram_tensor("user_attributes", [ROWS, D], f32, kind="ExternalInput").ap()
    v = nc.dram_tensor("image_attributes", [ROWS, D], f32, kind="ExternalInput").ap()
    ou = nc.dram_tensor("out_user", [ROWS, D], f32, kind="ExternalOutput").ap()
    ov = nc.dram_tensor("out_image", [ROWS, D], f32, kind="ExternalOutput").ap()

    with tile.TileContext(nc) as tc:
        with (
            tc.tile_pool(name="io", bufs=2) as io_pool,
            tc.tile_pool(name="sums", bufs=2) as sum_pool,
        ):
            for blk in range(N_BLOCKS):
                rows = slice(blk * P, (blk + 1) * P)

                ut = io_pool.tile([P, D], f32, tag="ut")
                nc.sync.dma_start(ut[:], u[rows, :])
                vt = io_pool.tile([P, D], f32, tag="vt")
                nc.sync.dma_start(vt[:], v[rows, :])

                us = sum_pool.tile([P, 1], f32, tag="us")
                nc.vector.reduce_sum(us[:], ut[:], axis=mybir.AxisListType.X)
                vs = sum_pool.tile([P, 1], f32, tag="vs")
                nc.vector.reduce_sum(vs[:], vt[:], axis=mybir.AxisListType.X)

                # out_user = user * img_sum on ACT (scaled copy),
                # out_image = image * usr_sum on DVE (2x-mode tensor_scalar).
                out_u = io_pool.tile([P, D], f32, tag="out_u")
                nc.scalar.activation(
                    out_u[:], ut[:], mybir.ActivationFunctionType.Copy, scale=vs[:]
                )
                out_v = io_pool.tile([P, D], f32, tag="out_v")
                nc.vector.tensor_scalar_mul(out_v[:], vt[:], us[:])

                nc.sync.dma_start(ou[rows, :], out_u[:])
                nc.sync.dma_start(ov[rows, :], out_v[:])

    nc.compile()
    return nc


def _get_runner():
    """Build + compile once; return a cached jitted sharded executor.

    Mirrors concourse.bass2jax.run_bass_via_pjrt's multi-core path, but
    keeps the jitted callable (and device-resident zero output buffers)
    across calls so repeat invocations skip retrace/recompile.
    """
    if "runner" in _CACHE:
        return _CACHE["runner"]

    import jax
    from jax.experimental.shard_map import shard_map
    from jax.sharding import Mesh, PartitionSpec

    from concourse import bass2jax, mybir

    nc = _build()
    bass2jax.install_neuronx_cc_hook()

    partition_name = nc.partition_id_tensor.name if nc.partition_id_tensor else None
    in_names, out_names, out_avals = [], [], []
    for alloc in nc.m.functions[0].allocations:
        if not isinstance(alloc, mybir.MemoryLocationSet):
            continue
        name = alloc.memorylocations[0].name
        if alloc.kind == "ExternalInput":
            if name != partition_name:
                in_names.append(name)
        elif alloc.kind == "ExternalOutput":
            out_names.append(name)
            out_avals.append(
                jax.core.ShapedArray(
                    tuple(alloc.tensor_shape), mybir.dt.np(alloc.dtype)
                )
            )
    all_in_names = list(in_names) + list(out_names)
    if partition_name is not None:
        all_in_names.append(partition_name)
    all_in_names = tuple(all_in_names)

    def _body(*args):
        operands = list(args)
        if partition_name is not None:
            operands.append(bass2jax.partition_id_tensor())
        outs = bass2jax._bass_exec_p.bind(
            *operands,
            out_avals=tuple(out_avals),
            in_names=all_in_names,
            out_names=tuple(out_names),
            lowering_input_output_aliases=(),
            sim_require_finite=True,
            sim_require_nnan=True,
            nc=nc,
        )
        return tuple(outs)

    devices = jax.devices()[:N_CORES]
    assert len(devices) == N_CORES
    mesh = Mesh(np.asarray(devices), ("core",))
    fn = jax.jit(
        shard_map(
            _body,
            mesh=mesh,
            in_specs=(PartitionSpec("core"),) * (len(in_names) + len(out_names)),
            out_specs=(PartitionSpec("core"),) * len(out_names),
            check_rep=False,
        ),
        keep_unused=True,
    )
    # Device-resident zero buffers handed to the custom call as the output
    # operands (not donated, so they stay valid across repeat calls). The
    # kernel writes every output element, so reuse is safe.
    zeros = [
        jax.device_put(np.zeros((a.shape[0] * N_CORES, *a.shape[1:]), a.dtype))
        for a in out_avals
    ]
    _CACHE["runner"] = (fn, in_names, out_names, zeros)
    return _CACHE["runner"]


def _prep(user_attributes, image_attributes):
    ua = np.ascontiguousarray(np.asarray(user_attributes, dtype=np.float32))
    ia = np.ascontiguousarray(np.asarray(image_attributes, dtype=np.float32))
    assert ua.shape == (B, D) and ia.shape == (B, D)
    return {"user_attributes": ua, "image_attributes": ia}


def kernel(user_attributes, image_attributes):
    fn, in_names, out_names, zeros = _get_runner()
    named = _prep(user_attributes, image_attributes)
    outs = fn(*[named[n] for n in in_names], *zeros)
    by_name = dict(zip(out_names, outs))
    return (
        np.asarray(by_name["out_user"]),
        np.asarray(by_name["out_image"]),
    )


# revision 8
# speedup vs baseline: 6158.6639x; 6158.6639x over previous
"""Trainium2 Bass kernel for nn_ExternalInteraction_9079560863791.

Computes, per batch row b:
    out_user[b, :]  = user_attributes[b, :]  * sum(image_attributes[b, :])
    out_image[b, :] = image_attributes[b, :] * sum(user_attributes[b, :])

Pure data parallel over the batch axis: 2048 rows split across 8 NeuronCores
(256 rows each). Per core: 2 blocks of 128 rows; each block loads a
[128, 4096] f32 tile per tensor, row-sums on the vector engine, and applies
the per-partition broadcast multiply (DVE tensor_scalar for one output, ACT
scaled-copy for the other, to spread compute across engines). Memory-bound:
~16 MiB of HBM traffic per core -> ~47 us roofline at ~358 GB/s.

`repeat` builds a NEFF that runs the whole pipeline N times back-to-back
(same inputs/outputs each pass) — used only for timing: the wall-clock slope
over N isolates steady-state per-execution device time from the large axon
dispatch overhead (~88 ms) that this container cannot profile away (no NTFF
hook in the bare axon RL image).
"""

import sys

for _p in ("/opt/trn_rl_repo", "/opt/pypackages"):
    if _p not in sys.path:
        sys.path.append(_p)

import numpy as np

N_CORES = 8
B, D = 2048, 4096
ROWS = B // N_CORES  # 256 rows per core
P = 128  # SBUF partitions
N_BLOCKS = ROWS // P  # 2 blocks per core

_CACHE = {}


def _build(repeat=1):
    import concourse.tile as tile
    from concourse import bacc, mybir

    nc = bacc.Bacc(
        "TRN2",
        target_bir_lowering=False,
        debug=False,
        enable_asserts=False,
        num_devices=N_CORES,
    )
    f32 = mybir.dt.float32

    u = nc.dram_tensor("user_attributes", [ROWS, D], f32, kind="ExternalInput").ap()
    v = nc.dram_tensor("image_attributes", [ROWS, D], f32, kind="ExternalInput").ap()
    ou = nc.dram_tensor("out_user", [ROWS, D], f32, kind="ExternalOutput").ap()
    ov = nc.dram_tensor("out_image", [ROWS, D], f32, kind="ExternalOutput").ap()

    with tile.TileContext(nc) as tc:
        with (
            tc.tile_pool(name="io", bufs=2) as io_pool,
            tc.tile_pool(name="sums", bufs=2) as sum_pool,
        ):
            for _rep in range(repeat):
                for blk in range(N_BLOCKS):
                    rows = slice(blk * P, (blk + 1) * P)

                    ut = io_pool.tile([P, D], f32, tag="ut")
                    nc.sync.dma_start(ut[:], u[rows, :])
                    vt = io_pool.tile([P, D], f32, tag="vt")
                    nc.sync.dma_start(vt[:], v[rows, :])

                    us = sum_pool.tile([P, 1], f32, tag="us")
                    nc.vector.reduce_sum(us[:], ut[:], axis=mybir.AxisListType.X)
                    vs = sum_pool.tile([P, 1], f32, tag="vs")
                    nc.vector.reduce_sum(vs[:], vt[:], axis=mybir.AxisListType.X)

                    # out_user = user * img_sum on ACT (scaled copy),
                    # out_image = image * usr_sum on DVE (2x tensor_scalar).
                    out_u = io_pool.tile([P, D], f32, tag="out_u")
                    nc.scalar.activation(
                        out_u[:], ut[:], mybir.ActivationFunctionType.Copy, scale=vs[:]
                    )
                    out_v = io_pool.tile([P, D], f32, tag="out_v")
                    nc.vector.tensor_scalar_mul(out_v[:], vt[:], us[:])

                    nc.sync.dma_start(ou[rows, :], out_u[:])
                    nc.sync.dma_start(ov[rows, :], out_v[:])

    nc.compile()
    return nc


def _make_runner(nc):
    """Jitted 8-core sharded executor for a compiled Bacc program. Mirrors
    concourse.bass2jax.run_bass_via_pjrt's multi-core path, but cached so
    repeat invocations skip retrace/recompile."""
    import jax
    from jax.experimental.shard_map import shard_map
    from jax.sharding import Mesh, PartitionSpec

    from concourse import bass2jax, mybir

    bass2jax.install_neuronx_cc_hook()

    partition_name = nc.partition_id_tensor.name if nc.partition_id_tensor else None
    in_names, out_names, out_avals = [], [], []
    for alloc in nc.m.functions[0].allocations:
        if not isinstance(alloc, mybir.MemoryLocationSet):
            continue
        name = alloc.memorylocations[0].name
        if alloc.kind == "ExternalInput":
            if name != partition_name:
                in_names.append(name)
        elif alloc.kind == "ExternalOutput":
            out_names.append(name)
            out_avals.append(
                jax.core.ShapedArray(
                    tuple(alloc.tensor_shape), mybir.dt.np(alloc.dtype)
                )
            )
    all_in_names = list(in_names) + list(out_names)
    if partition_name is not None:
        all_in_names.append(partition_name)
    all_in_names = tuple(all_in_names)

    def _body(*args):
        operands = list(args)
        if partition_name is not None:
            operands.append(bass2jax.partition_id_tensor())
        outs = bass2jax._bass_exec_p.bind(
            *operands,
            out_avals=tuple(out_avals),
            in_names=all_in_names,
            out_names=tuple(out_names),
            lowering_input_output_aliases=(),
            sim_require_finite=True,
            sim_require_nnan=True,
            nc=nc,
        )
        return tuple(outs)

    devices = jax.devices()[:N_CORES]
    assert len(devices) == N_CORES
    mesh = Mesh(np.asarray(devices), ("core",))
    fn = jax.jit(
        shard_map(
            _body,
            mesh=mesh,
            in_specs=(PartitionSpec("core"),) * (len(in_names) + len(out_names)),
            out_specs=(PartitionSpec("core"),) * len(out_names),
            check_rep=False,
        ),
        keep_unused=True,
    )
    return fn, in_names, out_names


def _get_runner(repeat=1):
    key = ("runner", repeat)
    if key not in _CACHE:
        _CACHE[key] = _make_runner(_build(repeat))
    return _CACHE[key]


def _prep(user_attributes, image_attributes):
    ua = np.ascontiguousarray(np.asarray(user_attributes, dtype=np.float32))
    ia = np.ascontiguousarray(np.asarray(image_attributes, dtype=np.float32))
    assert ua.shape == (B, D) and ia.shape == (B, D)
    return {"user_attributes": ua, "image_attributes": ia}


def kernel(user_attributes, image_attributes):
    import jax

    fn, in_names, out_names = _get_runner()
    if "zeros" not in _CACHE:
        # Output operands for the custom call (not donated, so they stay
        # valid across calls; the kernel writes every output element).
        _CACHE["zeros"] = [
            jax.device_put(np.zeros((B, D), np.float32)) for _ in out_names
        ]
    named = _prep(user_attributes, image_attributes)
    outs = fn(*[named[n] for n in in_names], *_CACHE["zeros"])
    by_name = dict(zip(out_names, outs))
    return (
        np.asarray(by_name["out_user"]),
        np.asarray(by_name["out_image"]),
    )
